# revision 8
# baseline (speedup 1.0000x reference)
"""Dice + contrastive loss on 8 Trainium2 NeuronCores.

Sharding: every input tensor [16,1,512,512] is flattened to [16, 262144]
and sharded along the *pixel* axis (32768 pixels per image per core).
Every term of the loss becomes a local partial reduction:

  - dice:   sum(sigmoid(pred)), sum(sigmoid(pred)*gt), sum(gt)   (scalars)
  - pos:    sum((mask*(s1-s2))^2) per image               (diag of a Gram)
  - sq1/2:  sum(s1^2), sum(s2^2) per image                (diag of a Gram)
  - cross:  s1 @ s2.T (16x16 Gram), contraction over pixels

Schedule (per core):
  - 7 large input DMAs (fp8 for sigmoid inputs + mask, bf16 for gt),
    ordered by consumer need; ~600-900ns HWDGE trigger cost each.
  - ACT spine: dummy sigmoid preloads the spline table under the DMA boot,
    then one instr per (in1_g|in2_g) pair written straight into Gram-pack
    layout, pred halves last (accum_out -> sum(p)), then PSUM evacuations.
  - DVE: d = s1-s2 (2x mode), dm = d*mask (1x, fp8 mask); sum(gt) and
    sum(p*gt) fused via tensor_tensor_reduce, placed in DVE slack slots.
  - PE: warm-up matmuls flip HAM to 2.4 GHz, then the 3 PSUM-accumulated
    Grams (psA = s1.[s1|s2], psB = s2.s2, psC = dm.dm) pipeline per pair.
  - tiny cross-core combine (a few KiB per core) happens on the host.
"""

import os
import sys

sys.path.insert(0, "/opt/trn_rl_repo")

import numpy as np
import ml_dtypes

import concourse.bass as bass
import concourse.tile as tile
from concourse import bacc, mybir
from concourse.bass_utils import run_bass_kernel_spmd

TAU = 0.1
DICE_SMOOTH = 0.1
WEIGHT = 1.0

NCORES = 8
B = 16                      # batch (images)
NPIX = 512 * 512            # pixels per image
PIX = NPIX // NCORES        # pixels per image per core = 32768
P = 128                     # partitions
F = PIX // P                # free columns per image per core = 256
T = 32                      # Gram contraction chunks (each covers 8 f-columns)
S = F // T                  # sub-columns per chunk = 8
G = 4                       # pair groups
TG = T // G                 # t-chunks per pair group = 8
BF = B * F                  # 4096

F32 = mybir.dt.float32
BF16 = mybir.dt.bfloat16
FP8 = mybir.dt.float8e4
NP_BF16 = ml_dtypes.bfloat16
NP_FP8 = ml_dtypes.float8_e4m3
AF = mybir.ActivationFunctionType
ALU = mybir.AluOpType

N_WARM = int(os.environ.get("N_WARM", "12"))


def _build_program():
    nc = bacc.Bacc("TRN2", target_bir_lowering=False, debug=False,
                   num_devices=NCORES)

    # x8 (fp8): [pair0 | pair1 | pair2 | pair3 | gt | pred]
    # x16 (bf16): [mask]
    d_x8 = nc.dram_tensor("x8", [P, 4 * 2048 + 2 * BF], FP8, kind="ExternalInput")
    d_x16 = nc.dram_tensor("x16", [P, BF], BF16, kind="ExternalInput")

    o_grams = nc.dram_tensor("grams", [P, 4 * P], F32, kind="ExternalOutput")
    o_stats = nc.dram_tensor("stats", [P, 6], F32, kind="ExternalOutput")

    with tile.TileContext(nc) as tc:
        with tc.tile_pool(name="main", bufs=1) as pool:
            t_pair = [pool.tile([P, 2048], FP8, name=f"t_pair{g}", tag=f"t_pair{g}")
                      for g in range(G)]
            t_mask = pool.tile([P, BF], BF16, tag="t_mask")
            t_gt = pool.tile([P, BF], FP8, tag="t_gt")
            gtr = pool.tile([P, 2048 + 1024 + 512], BF16, tag="gtr")
            t_pred = pool.tile([P, BF], FP8, tag="t_pred")
            # Gram-pack layout, col = t*256 + h*128 + (s*16+b); h=0: s1, h=1: s2
            s12 = pool.tile([P, 2 * BF], BF16, tag="s12")
            dd = pool.tile([P, 2 * BF], BF16, tag="dd")   # h=0: d, h=1: dm
            t_p = pool.tile([P, BF], BF16, tag="t_p")     # sigmoid(pred), natural
            t_scr = pool.tile([P, BF], BF16, tag="t_scr")  # reduce mandatory out
            stats = pool.tile([P, 6], F32, tag="stats")
            grams_sb = pool.tile([P, 4 * P], F32, tag="grams_sb")
            warm = pool.tile([P, 512], BF16, tag="warm")
            dum = pool.tile([P, 8], BF16, tag="dum")

            with tc.tile_pool(name="psum", bufs=1, space="PSUM") as psum_pool:
                psA = psum_pool.tile([P, 2 * P], F32, tag="psA")
                psB = psum_pool.tile([P, P], F32, tag="psB")
                psC = psum_pool.tile([P, P], F32, tag="psC")
                psW = psum_pool.tile([P, 512], F32, tag="psW")

                x8 = d_x8.ap()
                x16 = d_x16.ap()
                GT0 = 4 * 2048            # gt offset in x8
                PR0 = GT0 + BF            # pred offset in x8

                # ---- input DMAs (emission order = priority) ----
                nc.sync.dma_start(t_pair[0][:], x8[:, 0:2048])
                nc.sync.dma_start(t_mask[:, :2048], x16[:, 0:2048])
                nc.sync.dma_start(t_pair[1][:], x8[:, 2048:4096])
                nc.sync.dma_start(t_gt[:], x8[:, GT0:GT0 + BF])
                nc.sync.dma_start(t_mask[:, 2048:], x16[:, 2048:BF])
                nc.sync.dma_start(t_pair[2][:], x8[:, 4096:6144])
                nc.sync.dma_start(t_pred[:], x8[:, PR0:PR0 + BF])
                nc.sync.dma_start(t_pair[3][:], x8[:, 6144:8192])

                # ---- warm-up: ACT table preload + PE HAM unthrottle ----
                nc.vector.memset(dum[:], 0.0)
                nc.vector.memset(warm[:], 0.0)
                nc.scalar.activation(dum[:, 0:1], dum[:, 1:2], AF.Sigmoid)
                for i in range(N_WARM):
                    nc.tensor.matmul(psW[:], warm[:, :P], warm[:],
                                     start=True, stop=True)

                # gpsimd: sum(gt) tree-adds 4096 -> 512 (idle engine)
                nc.gpsimd.tensor_tensor(gtr[:, 0:2048], t_gt[:, 0:2048],
                                        t_gt[:, 2048:4096], ALU.add)
                nc.gpsimd.tensor_tensor(gtr[:, 2048:3072], gtr[:, 0:1024],
                                        gtr[:, 1024:2048], ALU.add)
                nc.gpsimd.tensor_tensor(gtr[:, 3072:3584], gtr[:, 2048:2560],
                                        gtr[:, 2560:3072], ALU.add)

                # s12/dd chunk views: [p, t, h, c]
                v_s12 = s12[:].rearrange("p (t h c) -> p t h c", h=2, c=P)
                v_dd = dd[:].rearrange("p (t h c) -> p t h c", h=2, c=P)
                v_mask = t_mask[:].rearrange("p (t c) -> p t c", c=P)
                s12r = s12[:]
                ddr = dd[:]

                def pred_half(h):
                    nc.scalar.activation(t_p[:, h * 2048:(h + 1) * 2048],
                                         t_pred[:, h * 2048:(h + 1) * 2048],
                                         AF.Sigmoid,
                                         accum_out=stats[:, h:h + 1])
                    nc.vector.scalar_tensor_tensor(
                        t_scr[:, h * 2048:(h + 1) * 2048],
                        t_p[:, h * 2048:(h + 1) * 2048], 0.0,
                        t_gt[:, h * 2048:(h + 1) * 2048],
                        ALU.bypass, ALU.mult,
                        accum_out=stats[:, 2 + h:3 + h])

                for g in range(G):
                    if g == 3:
                        pred_half(0)
                    # ACT: sigmoid of [in1_g | in2_g] into the pack layout
                    out_v = s12[:, g * 2048:(g + 1) * 2048].rearrange(
                        "p (t h c) -> p h t c", h=2, c=P)
                    nc.scalar.activation(out_v, t_pair[g][:], AF.Sigmoid)

                    ts = slice(g * TG, (g + 1) * TG)
                    # DVE: d = s1 - s2 ; dm = d * mask
                    nc.vector.tensor_tensor(v_dd[:, ts, 0, :],
                                            v_s12[:, ts, 0, :],
                                            v_s12[:, ts, 1, :], ALU.subtract)
                    nc.vector.tensor_tensor(v_dd[:, ts, 1, :],
                                            v_dd[:, ts, 0, :],
                                            v_mask[:, ts, :], ALU.mult)

                    # DVE slack slot: finish sum(gt) off the gpsimd tree
                    if g == 2:
                        nc.vector.scalar_tensor_tensor(
                            t_scr[:, :512], gtr[:, 3072:3584], 0.0,
                            gtr[:, 3072:3584], ALU.bypass, ALU.max,
                            accum_out=stats[:, 4:5])

                    # PE: Grams, PSUM-accumulated across all 32 chunks
                    for t in range(g * TG, (g + 1) * TG):
                        st = dict(start=(t == 0), stop=(t == T - 1))
                        c0, c1, c2 = t * 2 * P, t * 2 * P + P, (t + 1) * 2 * P
                        nc.tensor.matmul(psA[:], s12r[:, c0:c1], s12r[:, c0:c2], **st)
                        nc.tensor.matmul(psB[:], s12r[:, c1:c2], s12r[:, c1:c2], **st)
                        nc.tensor.matmul(psC[:], ddr[:, c1:c2], ddr[:, c1:c2], **st)


                pred_half(1)

                # ---- evacuate PSUM -> SBUF (all on scalar) -> DRAM ----
                nc.scalar.copy(grams_sb[:, :2 * P], psA[:])
                nc.scalar.copy(grams_sb[:, 2 * P:3 * P], psB[:])
                nc.scalar.copy(grams_sb[:, 3 * P:4 * P], psC[:])

                nc.sync.dma_start(o_grams.ap(), grams_sb[:])
                nc.sync.dma_start(o_stats.ap(), stats[:])

    nc.compile()
    return nc


_NC_CACHE = None


def _get_program():
    global _NC_CACHE
    if _NC_CACHE is None:
        _NC_CACHE = _build_program()
    return _NC_CACHE


def _shard_inputs(pred_labeled, gt_labeled, input1, input2, mask):
    flat = {
        "pred": np.asarray(pred_labeled, dtype=np.float32).reshape(B, NPIX),
        "gt": np.asarray(gt_labeled, dtype=np.float32).reshape(B, NPIX),
        "in1": np.asarray(input1, dtype=np.float32).reshape(B, NPIX),
        "in2": np.asarray(input2, dtype=np.float32).reshape(B, NPIX),
        "mask": np.asarray(mask, dtype=np.float32).reshape(B, NPIX),
    }

    def nat(a, sl):   # natural: [P, (b f)]
        return (a[:, sl].reshape(B, P, F).transpose(1, 0, 2)
                .reshape(P, B * F))

    def pack(a, sl):  # Gram pack: [P, (t s b)]
        return (a[:, sl].reshape(B, P, T, S).transpose(1, 2, 3, 0)
                .reshape(P, B * F))

    in_maps = []
    for k in range(NCORES):
        sl = slice(k * PIX, (k + 1) * PIX)
        pk1 = pack(flat["in1"], sl)
        pk2 = pack(flat["in2"], sl)
        x8 = np.empty((P, 4 * 2048 + 2 * BF), dtype=np.float32)
        for g in range(G):
            x8[:, g * 2048:g * 2048 + 1024] = pk1[:, g * 1024:(g + 1) * 1024]
            x8[:, g * 2048 + 1024:(g + 1) * 2048] = pk2[:, g * 1024:(g + 1) * 1024]
        x8[:, 4 * 2048:4 * 2048 + BF] = nat(flat["gt"], sl)
        x8[:, 4 * 2048 + BF:] = nat(flat["pred"], sl)
        in_maps.append({
            "x8": np.ascontiguousarray(x8).astype(NP_FP8),
            "x16": np.ascontiguousarray(pack(flat["mask"], sl)).astype(NP_BF16),
        })
    return in_maps


def _block_diag_sum(gmat):
    # [128, 128] with rows (s*16+b1), cols (s*16+b2) -> sum_s of [16,16] blocks
    g = gmat.reshape(S, B, S, B)
    return np.einsum("sbsc->bc", g)


def _combine(results):
    sum_p = sum_pg = sum_g = 0.0
    g1 = np.zeros((B, B), np.float64)
    cr = np.zeros((B, B), np.float64)
    g2 = np.zeros((B, B), np.float64)
    pc = np.zeros((B, B), np.float64)
    for r in results:
        st = r["stats"].astype(np.float64)
        sum_p += st[:, 0:2].sum()
        sum_pg += st[:, 2:4].sum()
        sum_g += st[:, 4:5].sum()
        gm = r["grams"].astype(np.float64)
        g1 += _block_diag_sum(gm[:, :P])
        cr += _block_diag_sum(gm[:, P:2 * P])
        g2 += _block_diag_sum(gm[:, 2 * P:3 * P])
        pc += _block_diag_sum(gm[:, 3 * P:4 * P])

    dice = 1.0 - (2.0 * sum_pg + DICE_SMOOTH) / (sum_p + sum_g + DICE_SMOOTH)

    n = float(NPIX)
    sq1 = np.diag(g1) / n
    sq2 = np.diag(g2) / n
    cross = cr / n
    pos_mse = np.diag(pc) / n

    sim_pos = np.exp(-pos_mse / TAU)
    mse = sq1[:, None] + sq2[None, :] - 2.0 * cross
    sim = np.exp(-mse / TAU)
    sim_neg = (sim * (1.0 - np.eye(B))).sum(axis=1)
    loss_c = float(np.mean(-np.log(sim_pos / (sim_pos + sim_neg))))
    total = dice + WEIGHT * loss_c
    return (np.float32(total), np.float32(dice), 0.0, np.float32(loss_c))


def kernel(pred_labeled, gt_labeled, input1, input2, mask):
    nc = _get_program()
    in_maps = _shard_inputs(pred_labeled, gt_labeled, input1, input2, mask)
    res = run_bass_kernel_spmd(nc, in_maps, core_ids=list(range(NCORES)),
                               trace=bool(int(os.environ.get("KERNEL_TRACE", "0"))))
    out = _combine(res.results)
    if res.exec_time_ns is not None:
        print(f"HW exec time: {res.exec_time_ns} ns")
    return out


# revision 9
# speedup vs baseline: 1.1642x; 1.1642x over previous
"""Dice + contrastive loss on 8 Trainium2 NeuronCores.

Sharding: every input tensor [16,1,512,512] is flattened to [16, 262144]
and sharded along the *pixel* axis (32768 pixels per image per core).
Every term of the loss becomes a local partial reduction:

  - dice:   sum(sigmoid(pred)), sum(sigmoid(pred)*gt), sum(gt)   (scalars)
  - pos:    sum((mask*(s1-s2))^2) per image               (diag of a Gram)
  - sq1/2:  sum(s1^2), sum(s2^2) per image                (diag of a Gram)
  - cross:  s1 @ s2.T (16x16 Gram), contraction over pixels

Schedule (per core):
  - 7 large input DMAs (fp8 for sigmoid inputs + mask, bf16 for gt),
    ordered by consumer need; ~600-900ns HWDGE trigger cost each.
  - ACT spine: dummy sigmoid preloads the spline table under the DMA boot,
    then one instr per (in1_g|in2_g) pair written straight into Gram-pack
    layout, pred halves last (accum_out -> sum(p)), then PSUM evacuations.
  - DVE: d = s1-s2 (2x mode), dm = d*mask (1x, fp8 mask); sum(gt) and
    sum(p*gt) fused via tensor_tensor_reduce, placed in DVE slack slots.
  - PE: warm-up matmuls flip HAM to 2.4 GHz, then the 3 PSUM-accumulated
    Grams (psA = s1.[s1|s2], psB = s2.s2, psC = dm.dm) pipeline per pair.
  - tiny cross-core combine (a few KiB per core) happens on the host.
"""

import os
import sys

sys.path.insert(0, "/opt/trn_rl_repo")

import numpy as np
import ml_dtypes

import concourse.bass as bass
import concourse.tile as tile
from concourse import bacc, mybir
from concourse.bass_utils import run_bass_kernel_spmd

TAU = 0.1
DICE_SMOOTH = 0.1
WEIGHT = 1.0

NCORES = 8
B = 16                      # batch (images)
NPIX = 512 * 512            # pixels per image
PIX = NPIX // NCORES        # pixels per image per core = 32768
P = 128                     # partitions
F = PIX // P                # free columns per image per core = 256
T = 32                      # Gram contraction chunks (each covers 8 f-columns)
S = F // T                  # sub-columns per chunk = 8
G = 4                       # pair groups
TG = T // G                 # t-chunks per pair group = 8
BF = B * F                  # 4096

F32 = mybir.dt.float32
BF16 = mybir.dt.bfloat16
FP8 = mybir.dt.float8e4
NP_BF16 = ml_dtypes.bfloat16
NP_FP8 = ml_dtypes.float8_e4m3
AF = mybir.ActivationFunctionType
ALU = mybir.AluOpType

N_WARM = int(os.environ.get("N_WARM", "12"))


def _build_program():
    nc = bacc.Bacc("TRN2", target_bir_lowering=False, debug=False,
                   num_devices=NCORES)

    # x8 (fp8): [pair0 | pair1 | pair2 | pair3 | gt | pred]
    # x16 (bf16): [mask]
    d_x8 = nc.dram_tensor("x8", [P, 4 * 2048 + 2 * BF], FP8, kind="ExternalInput")
    d_x16 = nc.dram_tensor("x16", [P, BF], BF16, kind="ExternalInput")

    o_grams = nc.dram_tensor("grams", [P, 4 * P + 8], F32, kind="ExternalOutput")

    with tile.TileContext(nc) as tc:
        with tc.tile_pool(name="main", bufs=1) as pool:
            t_pair = [pool.tile([P, 2048], FP8, name=f"t_pair{g}", tag=f"t_pair{g}")
                      for g in range(G)]
            t_mask = pool.tile([P, BF], BF16, tag="t_mask")
            t_gt = pool.tile([P, BF], FP8, tag="t_gt")
            gtr = pool.tile([P, 2048 + 1024 + 512], BF16, tag="gtr")
            # (gtr: sum(gt) DVE tree partials)
            t_pred = pool.tile([P, BF], FP8, tag="t_pred")
            # Gram-pack layout, col = t*256 + h*128 + (s*16+b); h=0: s1, h=1: s2
            s12 = pool.tile([P, 2 * BF], BF16, tag="s12")
            dd = pool.tile([P, 2 * BF], BF16, tag="dd")   # h=0: d, h=1: dm
            t_p = pool.tile([P, BF], BF16, tag="t_p")     # sigmoid(pred), natural
            t_scr = pool.tile([P, BF], BF16, tag="t_scr")  # reduce mandatory out
            grams_sb = pool.tile([P, 4 * P + 8], F32, tag="grams_sb")
            stats = grams_sb  # stats live in cols 512..519 of the output tile
            warm = pool.tile([P, 512], BF16, tag="warm")
            dum = pool.tile([P, 8], BF16, tag="dum")

            with tc.tile_pool(name="psum", bufs=1, space="PSUM") as psum_pool:
                psA = psum_pool.tile([P, 2 * P], F32, tag="psA")
                psB = psum_pool.tile([P, P], F32, tag="psB")
                psC = psum_pool.tile([P, P], F32, tag="psC")
                psW = psum_pool.tile([P, 512], F32, tag="psW")

                x8 = d_x8.ap()
                x16 = d_x16.ap()
                GT0 = 4 * 2048            # gt offset in x8
                PR0 = GT0 + BF            # pred offset in x8

                # ---- input DMAs (emission order = priority) ----
                nc.sync.dma_start(t_pair[0][:], x8[:, 0:2048])
                nc.sync.dma_start(t_pair[1][:], x8[:, 2048:4096])
                nc.sync.dma_start(t_mask[:, :2048], x16[:, 0:2048])
                nc.sync.dma_start(t_pair[2][:], x8[:, 4096:6144])
                nc.sync.dma_start(t_gt[:], x8[:, GT0:GT0 + BF])
                nc.sync.dma_start(t_pair[3][:], x8[:, 6144:8192])
                nc.sync.dma_start(t_pred[:, :2048], x8[:, PR0:PR0 + 2048])
                nc.sync.dma_start(t_pred[:, 2048:], x8[:, PR0 + 2048:PR0 + BF])
                nc.sync.dma_start(t_mask[:, 2048:], x16[:, 2048:BF])

                # ---- warm-up: ACT table preload + PE HAM unthrottle ----
                nc.vector.memset(dum[:], 0.0)
                nc.vector.memset(warm[:], 0.0)
                nc.scalar.activation(dum[:, 0:1], dum[:, 1:2], AF.Sigmoid)
                for i in range(N_WARM):
                    nc.tensor.matmul(psW[:], warm[:, :P], warm[:],
                                     start=True, stop=True)

                # s12/dd chunk views: [p, t, h, c]
                v_s12 = s12[:].rearrange("p (t h c) -> p t h c", h=2, c=P)
                v_dd = dd[:].rearrange("p (t h c) -> p t h c", h=2, c=P)
                v_mask = t_mask[:].rearrange("p (t c) -> p t c", c=P)
                s12r = s12[:]
                ddr = dd[:]

                def pred_half(h):
                    nc.scalar.activation(t_p[:, h * 2048:(h + 1) * 2048],
                                         t_pred[:, h * 2048:(h + 1) * 2048],
                                         AF.Sigmoid,
                                         accum_out=stats[:, 512 + h:513 + h])
                    nc.vector.scalar_tensor_tensor(
                        t_scr[:, h * 2048:(h + 1) * 2048],
                        t_p[:, h * 2048:(h + 1) * 2048], 0.0,
                        t_gt[:, h * 2048:(h + 1) * 2048],
                        ALU.bypass, ALU.mult,
                        accum_out=stats[:, 514 + h:515 + h])

                for g in range(G):
                    if g == 3:
                        pred_half(0)
                    # ACT: sigmoid of [in1_g | in2_g] into the pack layout
                    out_v = s12[:, g * 2048:(g + 1) * 2048].rearrange(
                        "p (t h c) -> p h t c", h=2, c=P)
                    nc.scalar.activation(out_v, t_pair[g][:], AF.Sigmoid)

                    ts = slice(g * TG, (g + 1) * TG)
                    # DVE: d = s1 - s2 ; dm = d * mask
                    nc.vector.tensor_tensor(v_dd[:, ts, 0, :],
                                            v_s12[:, ts, 0, :],
                                            v_s12[:, ts, 1, :], ALU.subtract)
                    nc.vector.tensor_tensor(v_dd[:, ts, 1, :],
                                            v_dd[:, ts, 0, :],
                                            v_mask[:, ts, :], ALU.mult)

                    # DVE slack slots: sum(gt) tree-adds 4096 -> 512
                    if g == 1:
                        nc.vector.tensor_tensor(gtr[:, 0:2048], t_gt[:, 0:2048],
                                                t_gt[:, 2048:4096], ALU.add)
                    if g == 2:
                        nc.vector.tensor_tensor(gtr[:, 2048:3072], gtr[:, 0:1024],
                                                gtr[:, 1024:2048], ALU.add)
                        nc.vector.tensor_tensor(gtr[:, 3072:3584],
                                                gtr[:, 2048:2560],
                                                gtr[:, 2560:3072], ALU.add)
                    if g == 3:
                        nc.vector.scalar_tensor_tensor(
                            t_scr[:, :512], gtr[:, 3072:3584], 0.0,
                            gtr[:, 3072:3584], ALU.bypass, ALU.max,
                            accum_out=stats[:, 516:517])

                    # PE: Grams, PSUM-accumulated across all 32 chunks
                    for t in range(g * TG, (g + 1) * TG):
                        st = dict(start=(t == 0), stop=(t == T - 1))
                        c0, c1, c2 = t * 2 * P, t * 2 * P + P, (t + 1) * 2 * P
                        nc.tensor.matmul(psA[:], s12r[:, c0:c1], s12r[:, c0:c2], **st)
                        nc.tensor.matmul(psB[:], s12r[:, c1:c2], s12r[:, c1:c2], **st)
                        nc.tensor.matmul(psC[:], ddr[:, c1:c2], ddr[:, c1:c2], **st)


                pred_half(1)

                # ---- evacuate PSUM -> SBUF (all on scalar) -> DRAM ----
                nc.scalar.copy(grams_sb[:, :2 * P], psA[:])
                nc.scalar.copy(grams_sb[:, 2 * P:3 * P], psB[:])
                nc.scalar.copy(grams_sb[:, 3 * P:4 * P], psC[:])

                nc.sync.dma_start(o_grams.ap(), grams_sb[:])

    nc.compile()
    return nc


_NC_CACHE = None


def _get_program():
    global _NC_CACHE
    if _NC_CACHE is None:
        _NC_CACHE = _build_program()
    return _NC_CACHE


def _shard_inputs(pred_labeled, gt_labeled, input1, input2, mask):
    flat = {
        "pred": np.asarray(pred_labeled, dtype=np.float32).reshape(B, NPIX),
        "gt": np.asarray(gt_labeled, dtype=np.float32).reshape(B, NPIX),
        "in1": np.asarray(input1, dtype=np.float32).reshape(B, NPIX),
        "in2": np.asarray(input2, dtype=np.float32).reshape(B, NPIX),
        "mask": np.asarray(mask, dtype=np.float32).reshape(B, NPIX),
    }

    def nat(a, sl):   # natural: [P, (b f)]
        return (a[:, sl].reshape(B, P, F).transpose(1, 0, 2)
                .reshape(P, B * F))

    def pack(a, sl):  # Gram pack: [P, (t s b)]
        return (a[:, sl].reshape(B, P, T, S).transpose(1, 2, 3, 0)
                .reshape(P, B * F))

    in_maps = []
    for k in range(NCORES):
        sl = slice(k * PIX, (k + 1) * PIX)
        pk1 = pack(flat["in1"], sl)
        pk2 = pack(flat["in2"], sl)
        x8 = np.empty((P, 4 * 2048 + 2 * BF), dtype=np.float32)
        for g in range(G):
            x8[:, g * 2048:g * 2048 + 1024] = pk1[:, g * 1024:(g + 1) * 1024]
            x8[:, g * 2048 + 1024:(g + 1) * 2048] = pk2[:, g * 1024:(g + 1) * 1024]
        x8[:, 4 * 2048:4 * 2048 + BF] = nat(flat["gt"], sl)
        x8[:, 4 * 2048 + BF:] = nat(flat["pred"], sl)
        in_maps.append({
            "x8": np.ascontiguousarray(x8).astype(NP_FP8),
            "x16": np.ascontiguousarray(pack(flat["mask"], sl)).astype(NP_BF16),
        })
    return in_maps


def _block_diag_sum(gmat):
    # [128, 128] with rows (s*16+b1), cols (s*16+b2) -> sum_s of [16,16] blocks
    g = gmat.reshape(S, B, S, B)
    return np.einsum("sbsc->bc", g)


def _combine(results):
    sum_p = sum_pg = sum_g = 0.0
    g1 = np.zeros((B, B), np.float64)
    cr = np.zeros((B, B), np.float64)
    g2 = np.zeros((B, B), np.float64)
    pc = np.zeros((B, B), np.float64)
    for r in results:
        gm = r["grams"].astype(np.float64)
        sum_p += gm[:, 512:514].sum()
        sum_pg += gm[:, 514:516].sum()
        sum_g += gm[:, 516:517].sum()
        g1 += _block_diag_sum(gm[:, :P])
        cr += _block_diag_sum(gm[:, P:2 * P])
        g2 += _block_diag_sum(gm[:, 2 * P:3 * P])
        pc += _block_diag_sum(gm[:, 3 * P:4 * P])

    dice = 1.0 - (2.0 * sum_pg + DICE_SMOOTH) / (sum_p + sum_g + DICE_SMOOTH)

    n = float(NPIX)
    sq1 = np.diag(g1) / n
    sq2 = np.diag(g2) / n
    cross = cr / n
    pos_mse = np.diag(pc) / n

    sim_pos = np.exp(-pos_mse / TAU)
    mse = sq1[:, None] + sq2[None, :] - 2.0 * cross
    sim = np.exp(-mse / TAU)
    sim_neg = (sim * (1.0 - np.eye(B))).sum(axis=1)
    loss_c = float(np.mean(-np.log(sim_pos / (sim_pos + sim_neg))))
    total = dice + WEIGHT * loss_c
    return (np.float32(total), np.float32(dice), 0.0, np.float32(loss_c))


def kernel(pred_labeled, gt_labeled, input1, input2, mask):
    nc = _get_program()
    in_maps = _shard_inputs(pred_labeled, gt_labeled, input1, input2, mask)
    res = run_bass_kernel_spmd(nc, in_maps, core_ids=list(range(NCORES)),
                               trace=bool(int(os.environ.get("KERNEL_TRACE", "0"))))
    out = _combine(res.results)
    if res.exec_time_ns is not None:
        print(f"HW exec time: {res.exec_time_ns} ns")
    return out
